# revision 31
# baseline (speedup 1.0000x reference)
"""Trainium2 Bass kernel for nn_Attention_86199993631321.

Reference computation (B=8, N=128, H=512):
    pair[b,i,j,:] = x[b,i,:] + x[b,j,:]
    out = pair @ W.T + b                # [B, N, N, H]

Key algebraic simplification: the Linear is applied to a *sum*, so
    out[b,i,j,:] = P[b,i,:] + P[b,j,:]   where P = x @ W.T + 0.5*b
This turns 68.7 GFLOP of einsum into a 0.5 GFLOP matmul plus a broadcast-add
that only has to *write* the 268 MB output.

Sharding: pure data-parallel over batch B (core b handles batch b), no
collectives.  Per core:
  - P = x_b @ W.T  via TensorE (inputs pre-transposed on host, packed into a
    single DRAM tensor).
  - P_rep = P + 0.5*b (bias via a DMA partition-broadcast tile), replicated
    4x along the free dim for later tensor_tensor reads.
  - P's rows are staged (cast to bf16) into the four legal PE row-group base
    partitions {0,32,64,96}; K=1 rank-1 matmuls ones ⊗ P[j,:] then run 4-way
    concurrently in the PE array via tile_position row groups.
  - VectorE adds P_rep + bcast (PSUM) -> bf16 SBUF out tiles.
  - Out tiles go to HBM as bf16 (half the write traffic); the host upcasts
    to f32.  The row permutation introduced by the quadrant packing is
    undone by the DMA access pattern (i (q s) o -> i (s q) o).
"""

import sys

if "/opt/trn_rl_repo" not in sys.path:
    sys.path.insert(0, "/opt/trn_rl_repo")

import numpy as np

B, N, H = 8, 128, 512
NCORES = 8
KC = H // 128  # contraction chunks for the P matmul
JBLK = 8       # j rows per output tile
TTW = 4        # j rows per PSUM tile / tensor_tensor op ([128, TTW*H] = 4 banks)
NQ = 4         # PE row-group quadrants
RPQ = JBLK // NQ  # rows per quadrant in a chunk (2)
# packed input layout (per core, bf16): wx[h, 0:128] = x.T,
# wx[h, 128:640] = W.T, wx[0, 640:768] = 1.0 (ones row for the bias matmul)
WXW = N + H + 128
# per-group eviction engine: the PE accumulates BOTH terms (rank-1 broadcast
# + identity matmul) into PSUM, so eviction is a single PSUM->bf16 copy.
# "S" = ScalarE activation-copy (~2.0us), "V" = VectorE tensor_copy (~2.3us);
# 19/13 split balances the two engines.
ROUTES = [
    "S", "V", "S", "S", "V", "S", "V", "S",
    "S", "V", "S", "S", "V", "S", "V", "S",
    "S", "V", "S", "S", "V", "S", "V", "S",
    "S", "V", "S", "S", "V", "S", "V", "S",
]

_BUILT = {}


def _build_nc():
    import concourse.bass as bass
    import concourse.bacc as bacc
    import concourse.tile as tile
    from concourse import mybir
    from concourse.masks import make_identity

    f32 = mybir.dt.float32
    bf16 = mybir.dt.bfloat16

    nc = bacc.Bacc()
    wx_ext = nc.declare_dram_parameter("wx", [H, WXW], bf16, isOutput=False)
    hb_ext = nc.declare_dram_parameter("halfb", [1, H], bf16, isOutput=False)
    out_ext = nc.declare_dram_parameter("out", [N, N, H], bf16, isOutput=True)

    with tile.TileContext(nc) as tc:
        with (
            tc.tile_pool(name="const", bufs=1) as const,
            tc.tile_pool(name="stage", bufs=4) as stage,
            tc.tile_pool(name="outp", bufs=3) as outp,
            tc.tile_pool(name="psum", bufs=2, space="PSUM") as psum,
        ):
            # ---- load packed inputs ----
            wx_sb = const.tile([128, KC, WXW], bf16)  # [h_local, (kc, m)]
            nc.sync.dma_start(
                out=wx_sb, in_=wx_ext.rearrange("(c p) m -> p c m", p=128)
            )
            # ones; slices at partitions {0,32,64,96} feed the four
            # concurrent row-group matmuls.
            ones_sb = const.tile([128, 128], bf16)
            nc.vector.memset(ones_sb, 1.0)
            hb_sb = const.tile([1, H], bf16)
            nc.sync.dma_start(out=hb_sb, in_=hb_ext[:, :])

            # ---- P = x @ W.T + 0.5*b -> PSUM [128(i), 512(o)] ----
            # bias folds in as a 5th K=1 matmul (ones row from wx ⊗ 0.5*b)
            ps_proj = psum.tile([128, TTW * H], f32, tag="ps")
            for c in range(KC):
                nc.tensor.matmul(
                    ps_proj[:, 0:H],
                    wx_sb[:, c, 0:N],
                    wx_sb[:, c, N : N + H],
                    start=(c == 0),
                    stop=False,
                )
            nc.tensor.matmul(
                ps_proj[:, 0:H],
                wx_sb[0:1, 0, N + H : N + H + 128],
                hb_sb,
                start=False,
                stop=True,
            )

            # P in bf16: rhs of the identity matmuls and source of the row
            # staging DMAs.
            P_bf = const.tile([128, H], bf16)
            nc.scalar.activation(
                P_bf, ps_proj[:, 0:H], mybir.ActivationFunctionType.Copy
            )
            # 128x128 identity (bf16) for the accumulate-P matmuls
            ident = const.tile([128, 128], bf16)
            make_identity(nc, ident)

            # ---- main loop over j-blocks ----
            for jt in range(N // JBLK):
                j0 = jt * JBLK
                # stage P rows into quadrants (bf16): partition 32*q holds
                # rows j0+2q, j0+2q+1 as [1, RPQ*H]
                # single strided-partition DMA: quadrant q (partition 32q)
                # receives rows j0+2q, j0+2q+1
                chunk = stage.tile([128, RPQ * H], bf16)
                nc.gpsimd.dma_start(
                    out=chunk[0:128:32, :],
                    in_=P_bf[j0 : j0 + JBLK, :],
                )
                out_tile = outp.tile([128, JBLK, H], bf16)
                # psum tile t covers rows j0+4t .. j0+4t+3 (quadrants 2t,2t+1,
                # rows in natural order).  Rank-1 broadcast matmuls are issued
                # q-fastest so all four PE row groups overlap; then identity
                # matmuls accumulate P into every slot (out = P_j + P_i).
                ps_a = psum.tile([128, TTW * H], f32, tag="ps")
                ps_b = psum.tile([128, TTW * H], f32, tag="ps")
                ps_tiles = [ps_a, ps_b]
                for s in range(RPQ):
                    for q in range(NQ):
                        ps_t = ps_tiles[q // 2]
                        slot = (q % 2) * RPQ + s  # local row within the tile
                        nc.tensor.matmul(
                            ps_t[:, slot * H : (slot + 1) * H],
                            ones_sb[q * 32 : q * 32 + 1, :],
                            chunk[q * 32 : q * 32 + 1, s * H : (s + 1) * H],
                            start=True,
                            stop=False,
                            tile_position=(q * 32, 0),
                        )
                for t, ps_t in enumerate(ps_tiles):
                    for slot in range(TTW):
                        nc.tensor.matmul(
                            ps_t[:, slot * H : (slot + 1) * H],
                            ident,
                            P_bf,
                            start=False,
                            stop=True,
                        )
                for t, ps_t in enumerate(ps_tiles):
                    route = ROUTES[(2 * jt + t) % len(ROUTES)]
                    out_sl = out_tile[:, t * TTW : (t + 1) * TTW, :]
                    ps_v = ps_t.rearrange("p (u h) -> p u h", u=TTW)
                    if route == "S":
                        nc.scalar.activation(
                            out_sl, ps_v, mybir.ActivationFunctionType.Copy
                        )
                    else:
                        nc.vector.tensor_copy(out_sl, ps_v)
                nc.sync.dma_start(
                    out=out_ext[:, j0 : j0 + JBLK, :], in_=out_tile
                )
    nc.compile()
    return nc


def _get_nc():
    if "nc" not in _BUILT:
        _BUILT["nc"] = _build_nc()
    return _BUILT["nc"]


def _make_in_maps(local_feats, W, b):
    import ml_dtypes

    bf = ml_dtypes.bfloat16
    local_feats = np.asarray(local_feats, dtype=np.float32)
    W = np.asarray(W, dtype=np.float32)
    b = np.asarray(b, dtype=np.float32)
    hb = np.ascontiguousarray((0.5 * b).reshape(1, H)).astype(bf)
    base = np.zeros((H, WXW), dtype=np.float32)
    base[:, N : N + H] = W.T
    base[0, N + H :] = 1.0
    in_maps = []
    for c in range(NCORES):
        wx = base.copy()
        wx[:, :N] = local_feats[c].T
        in_maps.append({"wx": wx.astype(bf), "halfb": hb})
    return in_maps


def _collect(res):
    return np.stack(
        [np.asarray(res.results[c]["out"]).astype(np.float32) for c in range(NCORES)],
        axis=0,
    )


def kernel(local_feats, W, b):
    from concourse.bass_utils import run_bass_kernel_spmd

    nc = _get_nc()
    in_maps = _make_in_maps(local_feats, W, b)
    res = run_bass_kernel_spmd(nc, in_maps, core_ids=list(range(NCORES)))
    return _collect(res)


def run_profiled(local_feats, W, b, **trace_kwargs):
    """Like kernel() but with neuron-profile tracing; returns (out, results)."""
    from concourse.bass_utils import run_bass_kernel_spmd

    nc = _get_nc()
    in_maps = _make_in_maps(local_feats, W, b)
    res = run_bass_kernel_spmd(
        nc, in_maps, core_ids=list(range(NCORES)), trace=True, **trace_kwargs
    )
    return _collect(res), res


# revision 35
# speedup vs baseline: 1.1377x; 1.1377x over previous
"""Trainium2 Bass kernel for nn_Attention_86199993631321.

Reference computation (B=8, N=128, H=512):
    pair[b,i,j,:] = x[b,i,:] + x[b,j,:]
    out = pair @ W.T + b                # [B, N, N, H]

Key algebraic simplification: the Linear is applied to a *sum*, so
    out[b,i,j,:] = P[b,i,:] + P[b,j,:]   where P = x @ W.T + 0.5*b
This turns 68.7 GFLOP of einsum into a 0.5 GFLOP matmul plus a broadcast-add
that only has to *write* the 268 MB output.

Sharding: pure data-parallel over batch B (core b handles batch b), no
collectives.  Per core:
  - P = x_b @ W.T  via TensorE (inputs pre-transposed on host, packed into a
    single DRAM tensor).
  - P_rep = P + 0.5*b (bias via a DMA partition-broadcast tile), replicated
    4x along the free dim for later tensor_tensor reads.
  - P's rows are staged (cast to bf16) into the four legal PE row-group base
    partitions {0,32,64,96}; K=1 rank-1 matmuls ones ⊗ P[j,:] then run 4-way
    concurrently in the PE array via tile_position row groups.
  - VectorE adds P_rep + bcast (PSUM) -> bf16 SBUF out tiles.
  - Out tiles go to HBM as bf16 (half the write traffic); the host upcasts
    to f32.  The row permutation introduced by the quadrant packing is
    undone by the DMA access pattern (i (q s) o -> i (s q) o).
"""

import sys

if "/opt/trn_rl_repo" not in sys.path:
    sys.path.insert(0, "/opt/trn_rl_repo")

import numpy as np

B, N, H = 8, 128, 512
NCORES = 8
KC = H // 128  # contraction chunks for the P matmul
JBLK = 8       # j rows per output tile
TTW = 4        # j rows per PSUM tile / tensor_tensor op ([128, TTW*H] = 4 banks)
NQ = 4         # PE row-group quadrants
RPQ = JBLK // NQ  # rows per quadrant in a chunk (2)
# packed input layout (per core, bf16): wx[h, 0:128] = x.T,
# wx[h, 128:640] = W.T, wx[0, 640:768] = 1.0 (ones row for the bias matmul)
WXW = N + H + 128
# per-group route (copy engine, TT engine).  Measured unit costs: ACT copy
# ~2.0us, DVE psum-copy ~2.3us, DVE bf16 TT ~1.2us, GpSimd bf16 TT ~4.1us.
# SV = ACT copy + DVE TT (18), SG = ACT copy + GpSimd TT (8), VV = DVE copy +
# DVE TT (6) -> ACT ~49us, DVE ~53us, GPS ~50us.
ROUTES = [
    "SV", "SG", "VV", "SV", "SV", "SG", "SV", "SV",
    "VV", "SV", "SG", "SV", "SV", "VV", "SG", "SV",
] * 2

_BUILT = {}


def _build_nc():
    import concourse.bass as bass
    import concourse.bacc as bacc
    import concourse.tile as tile
    from concourse import mybir

    f32 = mybir.dt.float32
    bf16 = mybir.dt.bfloat16

    nc = bacc.Bacc()
    wx_ext = nc.declare_dram_parameter("wx", [H, WXW], bf16, isOutput=False)
    hb_ext = nc.declare_dram_parameter("halfb", [1, H], bf16, isOutput=False)
    out_ext = nc.declare_dram_parameter("out", [N, N, H], bf16, isOutput=True)

    with tile.TileContext(nc) as tc:
        with (
            tc.tile_pool(name="const", bufs=1) as const,
            tc.tile_pool(name="stage", bufs=6) as stage,
            tc.tile_pool(name="bcast", bufs=8) as bcast,
            tc.tile_pool(name="outp", bufs=5) as outp,
            tc.tile_pool(name="psum", bufs=2, space="PSUM") as psum,
        ):
            # ---- load packed inputs ----
            wx_sb = const.tile([128, KC, WXW], bf16)  # [h_local, (kc, m)]
            nc.sync.dma_start(
                out=wx_sb, in_=wx_ext.rearrange("(c p) m -> p c m", p=128)
            )
            # ones; slices at partitions {0,32,64,96} feed the four
            # concurrent row-group matmuls.
            ones_sb = const.tile([128, 128], bf16)
            nc.vector.memset(ones_sb, 1.0)
            hb_sb = const.tile([1, H], bf16)
            nc.sync.dma_start(out=hb_sb, in_=hb_ext[:, :])

            # ---- P = x @ W.T + 0.5*b -> PSUM [128(i), 512(o)] ----
            # bias folds in as a 5th K=1 matmul (ones row from wx ⊗ 0.5*b)
            ps_proj = psum.tile([128, TTW * H], f32, tag="ps")
            for c in range(KC):
                nc.tensor.matmul(
                    ps_proj[:, 0:H],
                    wx_sb[:, c, 0:N],
                    wx_sb[:, c, N : N + H],
                    start=(c == 0),
                    stop=False,
                )
            nc.tensor.matmul(
                ps_proj[:, 0:H],
                wx_sb[0:1, 0, N + H : N + H + 128],
                hb_sb,
                start=False,
                stop=True,
            )

            # P_rep = P (bf16), replicated TTW times along the free dim.
            # bf16 keeps the tensor_tensor in the DVE 2x perf mode and lets
            # staging DMAs go over HWDGE (no cast).  Copies split ACT/DVE.
            P_rep = const.tile([128, TTW, H], bf16)
            for u in range(TTW):
                eng = nc.scalar if u % 2 == 0 else nc.vector
                if eng is nc.scalar:
                    nc.scalar.activation(
                        P_rep[:, u, :],
                        ps_proj[:, 0:H],
                        mybir.ActivationFunctionType.Copy,
                    )
                else:
                    nc.vector.tensor_copy(P_rep[:, u, :], ps_proj[:, 0:H])

            # ---- main loop over j-blocks ----
            for jt in range(N // JBLK):
                j0 = jt * JBLK
                # stage P rows into quadrants (bf16): partition 32*q holds
                # rows j0+2q, j0+2q+1 as [1, RPQ*H]
                # single strided-partition DMA: quadrant q (partition 32q)
                # receives rows j0+2q, j0+2q+1
                chunk = stage.tile([128, RPQ * H], bf16)
                nc.gpsimd.dma_start(
                    out=chunk[0:128:32, :],
                    in_=P_rep[j0 : j0 + JBLK, 0, :],
                )
                out_tile = outp.tile([128, JBLK, H], bf16)
                # psum tile t covers rows j0+4t .. j0+4t+3 (quadrants 2t,2t+1,
                # rows stay in natural order).  Matmuls are issued q-fastest so
                # all four PE row groups overlap in the array.
                ps_a = psum.tile([128, TTW * H], f32, tag="ps")
                ps_b = psum.tile([128, TTW * H], f32, tag="ps")
                ps_tiles = [ps_a, ps_b]
                for s in range(RPQ):
                    for q in range(NQ):
                        ps_t = ps_tiles[q // 2]
                        slot = (q % 2) * RPQ + s  # local row within the tile
                        nc.tensor.matmul(
                            ps_t[:, slot * H : (slot + 1) * H],
                            ones_sb[q * 32 : q * 32 + 1, :],
                            chunk[q * 32 : q * 32 + 1, s * H : (s + 1) * H],
                            start=True,
                            stop=True,
                            tile_position=(q * 32, 0),
                        )
                for t, ps_t in enumerate(ps_tiles):
                    route = ROUTES[(2 * jt + t) % len(ROUTES)]
                    out_sl = out_tile[:, t * TTW : (t + 1) * TTW, :]
                    # evict PSUM -> bf16 SBUF, then the add runs in 16-bit
                    # 2x mode on DVE (or on GpSimd)
                    bc_t = bcast.tile([128, TTW * H], bf16)
                    if route[0] == "S":
                        nc.scalar.activation(
                            bc_t, ps_t, mybir.ActivationFunctionType.Copy
                        )
                    else:
                        nc.vector.tensor_copy(bc_t, ps_t)
                    eng = nc.vector if route[1] == "V" else nc.gpsimd
                    eng.tensor_tensor(
                        out=out_sl,
                        in0=P_rep[:, :, :],
                        in1=bc_t.rearrange("p (u h) -> p u h", u=TTW),
                        op=mybir.AluOpType.add,
                    )
                nc.sync.dma_start(
                    out=out_ext[:, j0 : j0 + JBLK, :], in_=out_tile
                )
    nc.compile()
    return nc


def _get_nc():
    if "nc" not in _BUILT:
        _BUILT["nc"] = _build_nc()
    return _BUILT["nc"]


def _make_in_maps(local_feats, W, b):
    import ml_dtypes

    bf = ml_dtypes.bfloat16
    local_feats = np.asarray(local_feats, dtype=np.float32)
    W = np.asarray(W, dtype=np.float32)
    b = np.asarray(b, dtype=np.float32)
    hb = np.ascontiguousarray((0.5 * b).reshape(1, H)).astype(bf)
    base = np.zeros((H, WXW), dtype=np.float32)
    base[:, N : N + H] = W.T
    base[0, N + H :] = 1.0
    in_maps = []
    for c in range(NCORES):
        wx = base.copy()
        wx[:, :N] = local_feats[c].T
        in_maps.append({"wx": wx.astype(bf), "halfb": hb})
    return in_maps


def _collect(res):
    return np.stack(
        [np.asarray(res.results[c]["out"]).astype(np.float32) for c in range(NCORES)],
        axis=0,
    )


def kernel(local_feats, W, b):
    from concourse.bass_utils import run_bass_kernel_spmd

    nc = _get_nc()
    in_maps = _make_in_maps(local_feats, W, b)
    res = run_bass_kernel_spmd(nc, in_maps, core_ids=list(range(NCORES)))
    return _collect(res)


def run_profiled(local_feats, W, b, **trace_kwargs):
    """Like kernel() but with neuron-profile tracing; returns (out, results)."""
    from concourse.bass_utils import run_bass_kernel_spmd

    nc = _get_nc()
    in_maps = _make_in_maps(local_feats, W, b)
    res = run_bass_kernel_spmd(
        nc, in_maps, core_ids=list(range(NCORES)), trace=True, **trace_kwargs
    )
    return _collect(res), res
